# revision 1
# baseline (speedup 1.0000x reference)
"""HSTU block kernel for 8 Trainium2 NeuronCores.

Sharding: token-parallel. Core c handles batch b=c//4, tokens
[(c%4)*512, (c%4+1)*512). f1/attention/LN/f2 all computed locally for the
core's 512 query tokens; k/v for the full 2048-token batch are exchanged
with one AllGather per 4-core group.

Dataflow is feature-major (features on partitions) so the only transposes
are the initial x -> xT (32 PE transposes). LayerNorm over the feature dim
uses a ones-column matmul for the partition reduction and a K=1 ones-row
matmul to broadcast per-token stats back across partitions. The reference's
silu(scores)/S scaling is folded into LayerNorm via eps' = S^2 * eps
(LN is scale-invariant except for eps).

All big matmuls run in float32r (~13-bit mantissa, full PE rate).
"""

import sys

sys.path.insert(0, "/opt/trn_rl_repo")

import ml_dtypes
import numpy as np

import concourse.bass as bass
import concourse.mybir as mybir
import concourse.tile as tile
from concourse import bacc
from concourse.bass_utils import run_bass_kernel_spmd
from concourse.masks import make_identity

F32 = mybir.dt.float32
F32R = mybir.dt.float32r
BF16 = mybir.dt.bfloat16
SILU = mybir.ActivationFunctionType.Silu
SQRT = mybir.ActivationFunctionType.Sqrt
MULT = mybir.AluOpType.mult
ADD = mybir.AluOpType.add
SUB = mybir.AluOpType.subtract

B, S, D = 2, 2048, 1024
H, DH = 16, 64
T = 512            # tokens per core
NT = T // 128      # 4 token tiles per core
KC = D // 128      # 8 contraction chunks
NP = 8             # head pairs
EPS_EFF = float(S) * float(S) * 1e-5

_CACHE = {}


def _build():
    nc = bacc.Bacc(None, target_bir_lowering=False, num_devices=8)

    x_s = nc.dram_tensor("x_s", [T, D], F32, kind="ExternalInput")
    W1 = nc.dram_tensor("W1", [D, 4 * D], F32R, kind="ExternalInput")
    b1 = nc.dram_tensor("b1", [4 * D], F32, kind="ExternalInput")
    W2 = nc.dram_tensor("W2", [D, D], F32R, kind="ExternalInput")
    b2 = nc.dram_tensor("b2", [D], F32R, kind="ExternalInput")
    gamma = nc.dram_tensor("gamma", [D], F32, kind="ExternalInput")
    beta = nc.dram_tensor("beta", [D], F32, kind="ExternalInput")
    y_s = nc.dram_tensor("y_s", [T, D], F32, kind="ExternalOutput")

    # W1 column blocks: u [0:D], v [D:2D], q [2D:3D], k [3D:4D]
    U0, V0, Q0, K0 = 0, D, 2 * D, 3 * D

    with tile.TileContext(nc) as tc:
        with (
            tc.tile_pool(name="persist", bufs=1) as sbp,
            tc.tile_pool(name="small", bufs=2) as sbs,
            tc.tile_pool(name="dram", bufs=1, space="DRAM") as dram,
        ):
            # ---- constants
            ident = sbp.tile([128, 128], F32)
            make_identity(nc, ident[:])
            ones_f = sbp.tile([128, 128], F32)
            nc.vector.memset(ones_f[:], 1.0)
            ones_col = sbp.tile([128, 1], F32R)
            nc.vector.tensor_copy(ones_col[:], ones_f[:, 0:1])
            ones_row = sbp.tile([1, 128], F32R)
            nc.vector.tensor_copy(ones_row[:], ones_f[0:1, :])

            b1q = sbp.tile([128, 8], F32)
            b1k = sbp.tile([128, 8], F32)
            b1u = sbp.tile([128, 8], F32)
            nc.sync.dma_start(b1q[:], b1[Q0:Q0 + D].rearrange("(c p) -> p c", p=128))
            nc.sync.dma_start(b1k[:], b1[K0:K0 + D].rearrange("(c p) -> p c", p=128))
            nc.sync.dma_start(b1u[:], b1[U0:U0 + D].rearrange("(c p) -> p c", p=128))
            gam = sbp.tile([128, 8], F32)
            bet = sbp.tile([128, 8], F32)
            nc.sync.dma_start(gam[:], gamma[:].rearrange("(c p) -> p c", p=128))
            nc.sync.dma_start(bet[:], beta[:].rearrange("(c p) -> p c", p=128))

            b1v_row = sbp.tile([1, D], F32R)
            nc.sync.dma_start(b1v_row[:], b1[V0:V0 + D][None, :].bitcast(F32R))
            b2_row = sbp.tile([1, D], F32R)
            nc.sync.dma_start(b2_row[:], b2[:][None, :])

            # broadcast b1v / b2 across partitions via K=1 ones matmul
            b1v_sb = sbp.tile([128, D], F32)
            b2_sb = sbp.tile([128, D], F32)
            with tc.tile_pool(name="ps_bc", bufs=2, space="PSUM") as ps_bc:
                for nf in range(2):
                    pb = ps_bc.tile([128, 512], F32, tag="bc")
                    nc.tensor.matmul(pb[:], ones_row[:], b1v_row[:, nf * 512:(nf + 1) * 512],
                                     start=True, stop=True)
                    nc.vector.tensor_copy(b1v_sb[:, nf * 512:(nf + 1) * 512], pb[:])
                for nf in range(2):
                    pb = ps_bc.tile([128, 512], F32, tag="bc")
                    nc.tensor.matmul(pb[:], ones_row[:], b2_row[:, nf * 512:(nf + 1) * 512],
                                     start=True, stop=True)
                    nc.vector.tensor_copy(b2_sb[:, nf * 512:(nf + 1) * 512], pb[:])

            # ---- persistent activations
            xT = sbp.tile([128, KC, T], F32R)        # x^T, d on partitions
            qT = sbp.tile([128, NP, T], BF16)
            uT = sbp.tile([128, NP, T], F32)
            gatedT = sbp.tile([128, KC, T], F32R)
            normedT = sbp.tile([128, KC, T], F32R)

            # AG bounce buffers
            kv_in = dram.tile([128, 16, T], BF16)
            kv_out = dram.tile([512, 16, T], BF16)

            # ================= stage 0: load + transpose x =================
            with (
                tc.tile_pool(name="xload", bufs=2) as xload,
                tc.tile_pool(name="ps_tr", bufs=4, space="PSUM") as ps_tr,
            ):
                for tt in range(NT):
                    xa = xload.tile([128, D], F32, tag="xa")
                    nc.sync.dma_start(xa[:], x_s[tt * 128:(tt + 1) * 128, :])
                    for kc in range(KC):
                        pt = ps_tr.tile([128, 128], F32, tag="tr")
                        nc.tensor.transpose(pt[:], xa[:, kc * 128:(kc + 1) * 128], ident[:])
                        nc.vector.tensor_copy(xT[:, kc, tt * 128:(tt + 1) * 128], pt[:])

            # ================= stage 1: f1 =================
            with (
                tc.tile_pool(name="w1pool", bufs=12) as w1pool,
                tc.tile_pool(name="wvpool", bufs=2) as wvpool,
                tc.tile_pool(name="kvloc", bufs=1) as kvloc,
            ):
                kT_loc = kvloc.tile([128, NP, T], BF16)
                v_loc = kvloc.tile([128, NT, D], BF16)

                # k (feature-major) -> kT_loc
                with tc.tile_pool(name="ps_k", bufs=2, space="PSUM") as ps_k:
                  for hc in range(NP):
                    ps = ps_k.tile([128, T], F32, tag="f1")
                    for kc in range(KC):
                        wb = w1pool.tile([128, 128], F32R, tag="w1blk")
                        nc.sync.dma_start(
                            wb[:], W1[kc * 128:(kc + 1) * 128, K0 + hc * 128:K0 + (hc + 1) * 128])
                        nc.tensor.matmul(ps[:], wb[:], xT[:, kc, :],
                                         start=(kc == 0), stop=(kc == KC - 1))
                    nc.scalar.activation(kT_loc[:, hc, :], ps[:], SILU,
                                         bias=b1k[:, hc:hc + 1], scale=1.0)
                nc.gpsimd.dma_start(kv_in[:, 0:8, :], kT_loc[:])

                # v (token-major) -> v_loc; kc outer so each xT lhsT load
                # feeds both nf matmuls
                with tc.tile_pool(name="ps_v", bufs=1, space="PSUM") as ps_v:
                  psv = [ps_v.tile([128, 1024], F32, tag=f"v{tt}", name=f"psv{tt}")
                         for tt in range(NT)]
                  for kc in range(KC):
                    wv = wvpool.tile([128, 1024], F32R, tag="wv")
                    nc.sync.dma_start(wv[:], W1[kc * 128:(kc + 1) * 128, V0:V0 + D])
                    for tt in range(NT):
                        for nf in range(2):
                            nc.tensor.matmul(psv[tt][:, nf * 512:(nf + 1) * 512],
                                             xT[:, kc, tt * 128:(tt + 1) * 128],
                                             wv[:, nf * 512:(nf + 1) * 512],
                                             start=(kc == 0), stop=(kc == KC - 1))
                  for tt in range(NT):
                    vt = sbs.tile([128, 1024], F32, tag="vtmp")
                    nc.vector.tensor_tensor(vt[:], psv[tt][:], b1v_sb[:], ADD)
                    nc.scalar.activation(v_loc[:, tt, :], vt[:], SILU)
                nc.gpsimd.dma_start(
                    kv_in[:, 8:16, :],
                    v_loc[:].rearrange("p tt (h f) -> p (tt h) f", h=2))
                tc.no_sync_barrier()

                # q, u (overlap the AllGather)
                with tc.tile_pool(name="ps_qu", bufs=2, space="PSUM") as ps_qu:
                  for hc in range(NP):
                    ps = ps_qu.tile([128, T], F32, tag="f1")
                    for kc in range(KC):
                        wb = w1pool.tile([128, 128], F32R, tag="w1blk")
                        nc.sync.dma_start(
                            wb[:], W1[kc * 128:(kc + 1) * 128, Q0 + hc * 128:Q0 + (hc + 1) * 128])
                        nc.tensor.matmul(ps[:], wb[:], xT[:, kc, :],
                                         start=(kc == 0), stop=(kc == KC - 1))
                    nc.scalar.activation(qT[:, hc, :], ps[:], SILU,
                                         bias=b1q[:, hc:hc + 1], scale=1.0)
                  for hc in range(NP):
                    ps = ps_qu.tile([128, T], F32, tag="f1")
                    for kc in range(KC):
                        wb = w1pool.tile([128, 128], F32R, tag="w1blk")
                        nc.sync.dma_start(
                            wb[:], W1[kc * 128:(kc + 1) * 128, U0 + hc * 128:U0 + (hc + 1) * 128])
                        nc.tensor.matmul(ps[:], wb[:], xT[:, kc, :],
                                         start=(kc == 0), stop=(kc == KC - 1))
                    nc.scalar.activation(uT[:, hc, :], ps[:], SILU,
                                         bias=b1u[:, hc:hc + 1], scale=1.0)

                # single AllGather for k+v within each 4-core group
                nc.gpsimd.collective_compute(
                    "AllGather", mybir.AluOpType.bypass,
                    replica_groups=[[0, 1, 2, 3], [4, 5, 6, 7]],
                    ins=[kv_in[:]], outs=[kv_out[:]])

            # ================= stage 2: attention per head pair =================
            with (
                tc.tile_pool(name="kvfull", bufs=2) as kvfull,
                tc.tile_pool(name="attn", bufs=2) as attn,
                tc.tile_pool(name="ps_s", bufs=1, space="PSUM") as ps_s,
                tc.tile_pool(name="ps_av", bufs=2, space="PSUM") as ps_av,
            ):
                for hc in range(NP):
                    ktf = kvfull.tile([128, 2048], BF16, tag="ktf")
                    for r in range(4):
                        nc.sync.dma_start(ktf[:, r * 512:(r + 1) * 512],
                                          kv_out[r * 128:(r + 1) * 128, hc, :])
                    vf = kvfull.tile([128, 16, 128], BF16, tag="vf")
                    for r in range(4):
                        for tt in range(NT):
                            nc.sync.dma_start(
                                vf[:, r * 4 + tt, :],
                                kv_out[r * 128:(r + 1) * 128, 8 + tt * 2 + hc // 4,
                                       (hc % 4) * 128:(hc % 4) * 128 + 128])

                    av0 = ps_av.tile([128, 512], F32, tag="av0")
                    av1 = ps_av.tile([128, 512], F32, tag="av1")
                    for kg in range(8):
                        s0 = ps_s.tile([128, 1024], F32, tag="s0")
                        s1 = ps_s.tile([128, 1024], F32, tag="s1")
                        for sub in range(2):
                            ktc = kg * 2 + sub
                            nc.tensor.matmul(s0[:, sub * 512:(sub + 1) * 512],
                                             ktf[0:64, ktc * 128:(ktc + 1) * 128],
                                             qT[0:64, hc, :], start=True, stop=True)
                            nc.tensor.matmul(s1[:, sub * 512:(sub + 1) * 512],
                                             ktf[64:128, ktc * 128:(ktc + 1) * 128],
                                             qT[64:128, hc, :], start=True, stop=True,
                                             tile_position=(64, 0))
                        a0 = attn.tile([128, 1024], BF16, tag="a0")
                        a1 = attn.tile([128, 1024], BF16, tag="a1")
                        nc.scalar.activation(a0[:], s0[:], SILU)
                        nc.scalar.activation(a1[:], s1[:], SILU)
                        for sub in range(2):
                            ktc = kg * 2 + sub
                            # full-width lhsT: head0 valid rows 0:64, head1 rows 64:128
                            nc.tensor.matmul(av0[:], vf[:, ktc, :],
                                             a0[:, sub * 512:(sub + 1) * 512],
                                             start=(ktc == 0), stop=(ktc == 15))
                            nc.tensor.matmul(av1[:], vf[:, ktc, :],
                                             a1[:, sub * 512:(sub + 1) * 512],
                                             start=(ktc == 0), stop=(ktc == 15))
                    nc.vector.tensor_tensor(gatedT[0:64, hc, :], av0[0:64, :],
                                            uT[0:64, hc, :], MULT)
                    nc.vector.tensor_tensor(gatedT[64:128, hc, :], av1[64:128, :],
                                            uT[64:128, hc, :], MULT)

            # ================= stage 3: LayerNorm =================
            with (
                tc.tile_pool(name="ln", bufs=2) as ln,
                tc.tile_pool(name="ps_ln", bufs=1, space="PSUM") as ps_ln,
            ):
                st_sum = ps_ln.tile([1, T], F32, tag="st_sum")
                st_sq = ps_ln.tile([1, T], F32, tag="st_sq")
                for kc in range(KC):
                    nc.tensor.matmul(st_sum[:], ones_col[:], gatedT[:, kc, :],
                                     start=(kc == 0), stop=(kc == KC - 1))
                for kc in range(KC):
                    sq = ln.tile([128, T], F32R, tag="sq")
                    nc.vector.tensor_tensor(sq[:], gatedT[:, kc, :].bitcast(F32),
                                            gatedT[:, kc, :].bitcast(F32), MULT)
                    nc.tensor.matmul(st_sq[:], ones_col[:], sq[:],
                                     start=(kc == 0), stop=(kc == KC - 1))

                mu = ln.tile([1, T], F32, tag="mu")
                nc.vector.tensor_scalar_mul(mu[:], st_sum[:], 1.0 / D)
                m2 = ln.tile([1, T], F32, tag="m2")
                nc.vector.tensor_scalar_mul(m2[:], st_sq[:], 1.0 / D)
                mu2 = ln.tile([1, T], F32, tag="mu2")
                nc.vector.tensor_tensor(mu2[:], mu[:], mu[:], MULT)
                varE = ln.tile([1, T], F32, tag="varE")
                nc.vector.tensor_tensor(varE[:], m2[:], mu2[:], SUB)
                nc.vector.tensor_scalar_add(varE[:], varE[:], EPS_EFF)
                std = ln.tile([1, T], F32, tag="std")
                nc.scalar.activation(std[:], varE[:], SQRT)
                r0 = ln.tile([1, T], F32, tag="r0")
                nc.vector.reciprocal(r0[:], std[:])
                # one Newton step: r1 = r0 * (1.5 - 0.5 * varE * r0^2)
                nt1 = ln.tile([1, T], F32, tag="nt1")
                nc.vector.tensor_tensor(nt1[:], r0[:], r0[:], MULT)
                nc.vector.tensor_tensor(nt1[:], nt1[:], varE[:], MULT)
                nc.vector.tensor_scalar(nt1[:], nt1[:], -0.5, 1.5, MULT, ADD)
                rstd = ln.tile([1, T], F32R, tag="rstd")
                nc.vector.tensor_tensor(rstd[:], r0[:], nt1[:], MULT)
                mu_r = ln.tile([1, T], F32R, tag="mu_r")
                nc.vector.tensor_copy(mu_r[:], mu[:])

                ps_mu = ps_ln.tile([128, T], F32, tag="ps_mu")
                ps_r = ps_ln.tile([128, T], F32, tag="ps_r")
                nc.tensor.matmul(ps_mu[:], ones_row[:], mu_r[:], start=True, stop=True)
                nc.tensor.matmul(ps_r[:], ones_row[:], rstd[:], start=True, stop=True)

                for kc in range(KC):
                    t1 = ln.tile([128, T], F32, tag="t1")
                    nc.vector.tensor_tensor(t1[:], gatedT[:, kc, :].bitcast(F32), ps_mu[:], SUB)
                    nc.vector.tensor_tensor(t1[:], t1[:], ps_r[:], MULT)
                    nc.vector.tensor_scalar(normedT[:, kc, :], t1[:],
                                            gam[:, kc:kc + 1], bet[:, kc:kc + 1], MULT, ADD)

            # ================= stage 4: f2 + bias + store =================
            with (
                tc.tile_pool(name="w2pool", bufs=4) as w2pool,
                tc.tile_pool(name="yout", bufs=2) as yout,
                tc.tile_pool(name="ps_y", bufs=1, space="PSUM") as ps_y,
            ):
                for nf in range(2):
                    psy = [ps_y.tile([128, 512], F32, tag=f"y{tt}", name=f"psy{tt}") for tt in range(NT)]
                    for kc in range(KC):
                        w2b = w2pool.tile([128, 512], F32R, tag="w2b")
                        nc.sync.dma_start(
                            w2b[:], W2[kc * 128:(kc + 1) * 128, nf * 512:(nf + 1) * 512])
                        for tt in range(NT):
                            nc.tensor.matmul(psy[tt][:], normedT[:, kc, tt * 128:(tt + 1) * 128],
                                             w2b[:], start=(kc == 0), stop=(kc == KC - 1))
                    for tt in range(NT):
                        yo = yout.tile([128, 512], F32, tag="yo")
                        nc.vector.tensor_tensor(yo[:], psy[tt][:],
                                                b2_sb[:, nf * 512:(nf + 1) * 512], ADD)
                        nc.sync.dma_start(
                            y_s[tt * 128:(tt + 1) * 128, nf * 512:(nf + 1) * 512], yo[:])

    nc.compile()
    return nc


def _get_nc():
    if "nc" not in _CACHE:
        _CACHE["nc"] = _build()
    return _CACHE["nc"]


def kernel(x, W1, b1, W2, b2, gamma, beta, **kw):
    nc = _get_nc()
    x = np.ascontiguousarray(x, dtype=np.float32)
    in_maps = []
    for c in range(8):
        b = c // 4
        t0 = (c % 4) * T
        in_maps.append({
            "x_s": np.ascontiguousarray(x[b, t0:t0 + T, :]),
            "W1": np.ascontiguousarray(W1, dtype=np.float32),
            "b1": np.ascontiguousarray(b1, dtype=np.float32),
            "W2": np.ascontiguousarray(W2, dtype=np.float32),
            "b2": np.ascontiguousarray(b2, dtype=np.float32),
            "gamma": np.ascontiguousarray(gamma, dtype=np.float32),
            "beta": np.ascontiguousarray(beta, dtype=np.float32),
        })
    res = run_bass_kernel_spmd(nc, in_maps, core_ids=list(range(8)), **kw)
    y = np.empty((B, S, D), dtype=np.float32)
    for c in range(8):
        b = c // 4
        t0 = (c % 4) * T
        y[b, t0:t0 + T, :] = res.results[c]["y_s"]
    if kw:
        _CACHE["last_res"] = res
    return y



# revision 4
# speedup vs baseline: 1.8407x; 1.8407x over previous
"""HSTU block kernel for 8 Trainium2 NeuronCores — head-sharded (tensor parallel).

Sharding: core c owns heads {2c, 2c+1} (feature slice c*128:(c+1)*128 of each
of u/v/q/k) for BOTH batches, computing f1 + attention + gating for all 4096
tokens on its slice. A per-batch AllToAll (512KB) then redistributes the gated
activations token-wise (core c owns tokens [256c, 256c+256) of each batch) for
LayerNorm + f2. This replaces the old token-sharded design's 8MB k/v
AllGather (measured 128us on the ring) with 2x ~15us exchanges.

Everything runs in bf16 (PE rate on TRN2 is 1 row/cycle for bf16 and fp32r
alike, but bf16 halves DMA + LDWEIGHTS). x is pre-transposed and pre-cast on
the host, so the device does zero input transposes; v is transposed
feature->token major on the PE (32 small transposes) for the AV matmul.

silu(scores)/S scaling is folded into LayerNorm via eps' = S^2 * eps
(LN is scale-invariant except for eps). All activations (silu) run on the
ACT engine; LN's rsqrt uses ACT sqrt + DVE reciprocal + 1 Newton step,
placed after all silu work so the activation table loads only once.
"""

import sys

sys.path.insert(0, "/opt/trn_rl_repo")

import ml_dtypes
import numpy as np

import concourse.bass as bass
import concourse.mybir as mybir
import concourse.tile as tile
from concourse import bacc
from concourse.bass_utils import run_bass_kernel_spmd
from concourse.masks import make_identity

F32 = mybir.dt.float32
F32R = mybir.dt.float32r
BF16 = mybir.dt.bfloat16
SILU = mybir.ActivationFunctionType.Silu
SQRT = mybir.ActivationFunctionType.Sqrt
MULT = mybir.AluOpType.mult
ADD = mybir.AluOpType.add
SUB = mybir.AluOpType.subtract

B, S, D = 2, 2048, 1024
NT = B * S          # 4096 tokens total (both batches)
TOK = 256           # owned tokens per batch per core
KC = D // 128       # 8 contraction chunks
NKT = S // 128      # 16 k-token chunks per batch
EPS_EFF = float(S) * float(S) * 1e-5
BF = ml_dtypes.bfloat16

_CACHE = {}


def _build():
    nc = bacc.Bacc(None, target_bir_lowering=False, num_devices=8)

    x_t = nc.dram_tensor("x_t", [D, NT], BF16, kind="ExternalInput")
    w1c = nc.dram_tensor("w1c", [D, 512], BF16, kind="ExternalInput")
    b1c = nc.dram_tensor("b1c", [4, 128], F32, kind="ExternalInput")
    w2 = nc.dram_tensor("w2", [D, D], BF16, kind="ExternalInput")
    b2 = nc.dram_tensor("b2", [D], F32R, kind="ExternalInput")
    gamma = nc.dram_tensor("gamma", [D], F32, kind="ExternalInput")
    beta = nc.dram_tensor("beta", [D], F32, kind="ExternalInput")
    y_s = nc.dram_tensor("y_s", [B * TOK, D], F32, kind="ExternalOutput")

    with tile.TileContext(nc) as tc:
        with (
            tc.tile_pool(name="persist", bufs=1) as sbp,
            tc.tile_pool(name="small", bufs=2) as sbs,
            tc.tile_pool(name="dram", bufs=1, space="DRAM") as dram,
        ):
            # ---- persistent SBUF
            xT = sbp.tile([128, KC, NT], BF16)
            w1sb = sbp.tile([128, KC, 512], BF16)
            w2sb = sbp.tile([128, KC, D], BF16)
            kT = sbp.tile([128, NT], BF16)
            vT = sbp.tile([128, NT], BF16)
            qT = sbp.tile([128, NT], BF16)
            uT = sbp.tile([128, NT], BF16)
            vtok = sbp.tile([128, 2 * NKT, 128], BF16)
            gatedT = sbp.tile([128, NT], BF16)
            gfull = sbp.tile([128, KC, B * TOK], BF16)
            normed = sbp.tile([128, KC, B * TOK], BF16)

            # ---- input DMAs (order: first-needed first)
            nc.sync.dma_start(w1sb[:], w1c[:].rearrange("(kc p) f -> p kc f", p=128))
            for kc in range(KC):
                nc.sync.dma_start(xT[:, kc, :], x_t[kc * 128:(kc + 1) * 128, :])
            b1sb = sbp.tile([128, 4], F32)
            nc.sync.dma_start(b1sb[:], b1c[:].rearrange("c p -> p c"))
            gam = sbp.tile([128, KC], F32)
            bet = sbp.tile([128, KC], F32)
            nc.sync.dma_start(gam[:], gamma[:].rearrange("(c p) -> p c", p=128))
            nc.sync.dma_start(bet[:], beta[:].rearrange("(c p) -> p c", p=128))
            b2_row = sbp.tile([1, D], F32R)
            nc.sync.dma_start(b2_row[:], b2[:][None, :])
            nc.sync.dma_start(w2sb[:], w2[:].rearrange("(kc p) f -> p kc f", p=128))

            # ---- constants
            ident_bf = sbp.tile([128, 128], BF16)
            make_identity(nc, ident_bf[:])
            ones_f = sbp.tile([128, 128], F32)
            nc.vector.memset(ones_f[:], 1.0)
            ones_col_bf = sbp.tile([128, 1], BF16)
            nc.vector.tensor_copy(ones_col_bf[:], ones_f[:, 0:1])
            ones_row_r = sbp.tile([1, 128], F32R)
            nc.vector.tensor_copy(ones_row_r[:], ones_f[0:1, :])

            # broadcast b2 across partitions via K=1 ones matmul
            b2sb = sbp.tile([128, D], F32)
            with tc.tile_pool(name="ps_bc", bufs=2, space="PSUM") as ps_bc:
                for nf in range(2):
                    pb = ps_bc.tile([128, 512], F32, tag="bc")
                    nc.tensor.matmul(pb[:], ones_row_r[:],
                                     b2_row[:, nf * 512:(nf + 1) * 512],
                                     start=True, stop=True)
                    nc.vector.tensor_copy(b2sb[:, nf * 512:(nf + 1) * 512], pb[:])

            # a2a DRAM bounce buffers (per batch)
            a2a_in = [dram.tile([D, TOK], BF16, name=f"a2a_in{b}")
                      for b in range(B)]
            a2a_out = [dram.tile([D, TOK], BF16, name=f"a2a_out{b}")
                       for b in range(B)]

            # ================= f1: k, v (feature-major, all 4096 tokens) ====
            PROJ_OUT = [kT, vT, qT, uT]

            def f1_grp(ps_pool, proj, grp):
                ps = ps_pool.tile([128, 2048], F32, tag="f1")
                for kc in range(KC):
                    lhsT = w1sb[:, kc, proj * 128:(proj + 1) * 128]
                    for t4 in range(4):
                        o = grp * 2048 + t4 * 512
                        nc.tensor.matmul(ps[:, t4 * 512:(t4 + 1) * 512],
                                         lhsT, xT[:, kc, o:o + 512],
                                         start=(kc == 0), stop=(kc == KC - 1))
                nc.scalar.activation(
                    PROJ_OUT[proj][:, grp * 2048:(grp + 1) * 2048], ps[:], SILU,
                    bias=b1sb[:, proj:proj + 1], scale=1.0)

            with tc.tile_pool(name="ps_f1a", bufs=2, space="PSUM") as ps_f1:
                for proj in (0, 1):           # k, v
                    for grp in (0, 1):
                        f1_grp(ps_f1, proj, grp)

            # ================= v transpose: [f, tok] -> [tok, f] =============
            with tc.tile_pool(name="ps_tr", bufs=4, space="PSUM") as ps_tr:
                for i in range(2 * NKT):
                    pt = ps_tr.tile([128, 128], BF16, tag="tr")
                    nc.tensor.transpose(pt[:], vT[:, i * 128:(i + 1) * 128],
                                        ident_bf[:])
                    nc.vector.tensor_copy(vtok[:, i, :], pt[:])

            # ================= f1: q, u =====================================
            with tc.tile_pool(name="ps_f1b", bufs=2, space="PSUM") as ps_f1b:
                for proj in (2, 3):           # q, u
                    for grp in (0, 1):
                        f1_grp(ps_f1b, proj, grp)

            # ================= attention per batch ==========================
            with (
                tc.tile_pool(name="attn_a", bufs=3) as attn_a,
                tc.tile_pool(name="ps_s", bufs=3, space="PSUM") as ps_s,
                tc.tile_pool(name="ps_av", bufs=2, space="PSUM") as ps_av,
            ):
                for b in range(B):
                    boff = b * S
                    for qc in range(4):
                        qsl = boff + qc * 512
                        av = ps_av.tile([128, 512], F32, tag="av")
                        s_tiles = {}
                        a_tiles = {}

                        def emit_s(ktc):
                            s = ps_s.tile([128, 1024], F32, tag="s")
                            ksl = boff + ktc * 128
                            nc.tensor.matmul(s[:, 0:512],
                                             kT[0:64, ksl:ksl + 128],
                                             qT[0:64, qsl:qsl + 512],
                                             start=True, stop=True)
                            nc.tensor.matmul(s[:, 512:1024],
                                             kT[64:128, ksl:ksl + 128],
                                             qT[64:128, qsl:qsl + 512],
                                             start=True, stop=True,
                                             tile_position=(64, 0))
                            a = attn_a.tile([128, 1024], BF16, tag="a")
                            nc.scalar.activation(a[:], s[:], SILU)
                            a_tiles[ktc] = a

                        def emit_av(ktc):
                            a = a_tiles.pop(ktc)
                            vc = b * NKT + ktc
                            nc.tensor.matmul(av[0:64, :], vtok[:, vc, 0:64],
                                             a[:, 0:512],
                                             start=(ktc == 0), stop=(ktc == NKT - 1))
                            nc.tensor.matmul(av[64:128, :], vtok[:, vc, 64:128],
                                             a[:, 512:1024],
                                             start=(ktc == 0), stop=(ktc == NKT - 1),
                                             tile_position=(0, 64))

                        for ktc in range(NKT):
                            emit_s(ktc)
                            if ktc >= 1:
                                emit_av(ktc - 1)
                        emit_av(NKT - 1)

                        nc.vector.tensor_tensor(gatedT[:, qsl:qsl + 512], av[:],
                                                uT[:, qsl:qsl + 512], MULT)

                    # ship gated slice of this batch: dest d gets tokens
                    # [d*TOK, (d+1)*TOK) of batch b
                    for d in range(8):
                        nc.gpsimd.dma_start(
                            a2a_in[b][d * 128:(d + 1) * 128, :],
                            gatedT[:, boff + d * TOK: boff + (d + 1) * TOK])
                    nc.gpsimd.collective_compute(
                        "AllToAll", mybir.AluOpType.bypass,
                        replica_groups=[[0, 1, 2, 3, 4, 5, 6, 7]],
                        ins=[a2a_in[b][:]], outs=[a2a_out[b][:]])
                    # readback: block s holds features [s*128,(s+1)*128) of
                    # my TOK tokens (overlaps next batch's attention)
                    for sblk in range(8):
                        nc.sync.dma_start(
                            gfull[:, sblk, b * TOK:(b + 1) * TOK],
                            a2a_out[b][sblk * 128:(sblk + 1) * 128, :])

            # ================= LayerNorm + f2 per batch =====================
            with (
                tc.tile_pool(name="ln", bufs=2) as ln,
                tc.tile_pool(name="sqp", bufs=1) as sqp,
                tc.tile_pool(name="yout", bufs=2) as yout,
                tc.tile_pool(name="ps_ln", bufs=1, space="PSUM") as ps_ln,
                tc.tile_pool(name="ps_y", bufs=2, space="PSUM") as ps_y,
            ):
                for b in range(B):
                    bsl = slice(b * TOK, (b + 1) * TOK)
                    sq = sqp.tile([128, KC, TOK], BF16, tag="sq")
                    for kc in range(KC):
                        nc.vector.tensor_tensor(sq[:, kc, :], gfull[:, kc, bsl],
                                                gfull[:, kc, bsl], MULT)
                    st_sum = ps_ln.tile([1, TOK], F32, tag="st_sum")
                    st_sq = ps_ln.tile([1, TOK], F32, tag="st_sq")
                    for kc in range(KC):
                        nc.tensor.matmul(st_sum[:], ones_col_bf[:],
                                         gfull[:, kc, bsl],
                                         start=(kc == 0), stop=(kc == KC - 1))
                    for kc in range(KC):
                        nc.tensor.matmul(st_sq[:], ones_col_bf[:], sq[:, kc, :],
                                         start=(kc == 0), stop=(kc == KC - 1))

                    mu = ln.tile([1, TOK], F32, tag="mu")
                    nc.vector.tensor_scalar_mul(mu[:], st_sum[:], 1.0 / D)
                    m2 = ln.tile([1, TOK], F32, tag="m2")
                    nc.vector.tensor_scalar_mul(m2[:], st_sq[:], 1.0 / D)
                    mu2 = ln.tile([1, TOK], F32, tag="mu2")
                    nc.vector.tensor_tensor(mu2[:], mu[:], mu[:], MULT)
                    varE = ln.tile([1, TOK], F32, tag="varE")
                    nc.vector.tensor_tensor(varE[:], m2[:], mu2[:], SUB)
                    nc.vector.tensor_scalar_add(varE[:], varE[:], EPS_EFF)
                    std = ln.tile([1, TOK], F32, tag="std")
                    nc.scalar.activation(std[:], varE[:], SQRT)
                    r0 = ln.tile([1, TOK], F32, tag="r0")
                    nc.vector.reciprocal(r0[:], std[:])
                    # one Newton step: r1 = r0 * (1.5 - 0.5 * varE * r0^2)
                    nt1 = ln.tile([1, TOK], F32, tag="nt1")
                    nc.vector.tensor_tensor(nt1[:], r0[:], r0[:], MULT)
                    nc.vector.tensor_tensor(nt1[:], nt1[:], varE[:], MULT)
                    nc.vector.tensor_scalar(nt1[:], nt1[:], -0.5, 1.5, MULT, ADD)
                    rstd = ln.tile([1, TOK], F32R, tag="rstd")
                    nc.vector.tensor_tensor(rstd[:], r0[:], nt1[:], MULT)
                    mu_r = ln.tile([1, TOK], F32R, tag="mu_r")
                    nc.vector.tensor_copy(mu_r[:], mu[:])

                    ps_mu = ps_ln.tile([128, TOK], F32, tag="ps_mu")
                    ps_r = ps_ln.tile([128, TOK], F32, tag="ps_r")
                    nc.tensor.matmul(ps_mu[:], ones_row_r[:], mu_r[:],
                                     start=True, stop=True)
                    nc.tensor.matmul(ps_r[:], ones_row_r[:], rstd[:],
                                     start=True, stop=True)

                    for kc in range(KC):
                        t1 = ln.tile([128, TOK], F32, tag="t1")
                        nc.vector.tensor_tensor(t1[:], gfull[:, kc, bsl],
                                                ps_mu[:], SUB)
                        nc.vector.tensor_tensor(t1[:], t1[:], ps_r[:], MULT)
                        nc.vector.tensor_scalar(normed[:, kc, bsl], t1[:],
                                                gam[:, kc:kc + 1],
                                                bet[:, kc:kc + 1], MULT, ADD)

                    # f2: y[tok, :] = normed^T @ W2 + b2
                    for tt in range(TOK // 128):
                        for nf in range(2):
                            psy = ps_y.tile([128, 512], F32, tag="y")
                            for kc in range(KC):
                                nc.tensor.matmul(
                                    psy[:],
                                    normed[:, kc, b * TOK + tt * 128:
                                           b * TOK + (tt + 1) * 128],
                                    w2sb[:, kc, nf * 512:(nf + 1) * 512],
                                    start=(kc == 0), stop=(kc == KC - 1))
                            yo = yout.tile([128, 512], F32, tag="yo")
                            nc.vector.tensor_tensor(
                                yo[:], psy[:], b2sb[:, nf * 512:(nf + 1) * 512],
                                ADD)
                            nc.sync.dma_start(
                                y_s[b * TOK + tt * 128: b * TOK + (tt + 1) * 128,
                                    nf * 512:(nf + 1) * 512], yo[:])

    nc.compile()
    return nc


def _get_nc():
    if "nc" not in _CACHE:
        _CACHE["nc"] = _build()
    return _CACHE["nc"]


def kernel(x, W1, b1, W2, b2, gamma, beta, **kw):
    nc = _get_nc()
    x = np.ascontiguousarray(x, dtype=np.float32)
    W1 = np.ascontiguousarray(W1, dtype=np.float32)
    U0, V0, Q0, K0 = 0, D, 2 * D, 3 * D

    x2 = np.concatenate([x[0], x[1]], axis=0)            # [4096, 1024]
    xt = np.ascontiguousarray(x2.T.astype(BF))           # [1024, 4096]
    w2_bf = np.ascontiguousarray(W2.astype(BF))

    in_maps = []
    for c in range(8):
        sl = slice(c * 128, (c + 1) * 128)
        w1c = np.ascontiguousarray(np.concatenate(
            [W1[:, K0 + c * 128:K0 + (c + 1) * 128],
             W1[:, V0 + c * 128:V0 + (c + 1) * 128],
             W1[:, Q0 + c * 128:Q0 + (c + 1) * 128],
             W1[:, U0 + c * 128:U0 + (c + 1) * 128]], axis=1).astype(BF))
        b1cc = np.ascontiguousarray(np.stack(
            [b1[K0 + c * 128:K0 + (c + 1) * 128],
             b1[V0 + c * 128:V0 + (c + 1) * 128],
             b1[Q0 + c * 128:Q0 + (c + 1) * 128],
             b1[U0 + c * 128:U0 + (c + 1) * 128]], axis=0).astype(np.float32))
        in_maps.append({
            "x_t": xt,
            "w1c": w1c,
            "b1c": b1cc,
            "w2": w2_bf,
            "b2": np.ascontiguousarray(b2, dtype=np.float32),
            "gamma": np.ascontiguousarray(gamma, dtype=np.float32),
            "beta": np.ascontiguousarray(beta, dtype=np.float32),
        })
    res = run_bass_kernel_spmd(nc, in_maps, core_ids=list(range(8)), **kw)
    y = np.empty((B, S, D), dtype=np.float32)
    for c in range(8):
        for b in range(B):
            y[b, c * TOK:(c + 1) * TOK, :] = \
                res.results[c]["y_s"][b * TOK:(b + 1) * TOK]
    if kw:
        _CACHE["last_res"] = res
    return y


# revision 10
# speedup vs baseline: 1.8436x; 1.0016x over previous
"""HSTU block kernel for 8 Trainium2 NeuronCores — head-sharded (tensor parallel).

Sharding: core c owns heads {2c, 2c+1} (feature slice c*128:(c+1)*128 of each
of u/v/q/k) for BOTH batches, computing f1 + attention + gating for all 4096
tokens on its slice. A per-batch AllToAll (512KB) then redistributes the gated
activations token-wise (core c owns tokens [256c, 256c+256) of each batch) for
LayerNorm + f2. This replaces the old token-sharded design's 8MB k/v
AllGather (measured 128us on the ring) with 2x ~15us exchanges.

Everything runs in bf16 (PE rate on TRN2 is 1 row/cycle for bf16 and fp32r
alike, but bf16 halves DMA + LDWEIGHTS). x is pre-transposed and pre-cast on
the host, so the device does zero input transposes; v is transposed
feature->token major on the PE (32 small transposes) for the AV matmul.

silu(scores)/S scaling is folded into LayerNorm via eps' = S^2 * eps
(LN is scale-invariant except for eps). All activations (silu) run on the
ACT engine; LN's rsqrt uses ACT sqrt + DVE reciprocal + 1 Newton step,
placed after all silu work so the activation table loads only once.
"""

import sys

sys.path.insert(0, "/opt/trn_rl_repo")

import ml_dtypes
import numpy as np

import concourse.bass as bass
import concourse.mybir as mybir
import concourse.tile as tile
from concourse import bacc
from concourse.bass_utils import run_bass_kernel_spmd
from concourse.masks import make_identity

F32 = mybir.dt.float32
F32R = mybir.dt.float32r
BF16 = mybir.dt.bfloat16
SILU = mybir.ActivationFunctionType.Silu
SQRT = mybir.ActivationFunctionType.Sqrt
MULT = mybir.AluOpType.mult
ADD = mybir.AluOpType.add
SUB = mybir.AluOpType.subtract

B, S, D = 2, 2048, 1024
NT = B * S          # 4096 tokens total (both batches)
TOK = 256           # owned tokens per batch per core
KC = D // 128       # 8 contraction chunks
NKT = S // 128      # 16 k-token chunks per batch
EPS_EFF = float(S) * float(S) * 1e-5
BF = ml_dtypes.bfloat16

_CACHE = {}


def _build():
    nc = bacc.Bacc(None, target_bir_lowering=False, num_devices=8)

    x_t = nc.dram_tensor("x_t", [D, NT], BF16, kind="ExternalInput")
    w1c = nc.dram_tensor("w1c", [D, 512], BF16, kind="ExternalInput")
    b1c = nc.dram_tensor("b1c", [4, 128], F32, kind="ExternalInput")
    w2 = nc.dram_tensor("w2", [D, D], BF16, kind="ExternalInput")
    b2 = nc.dram_tensor("b2", [D], F32R, kind="ExternalInput")
    gamma = nc.dram_tensor("gamma", [D], F32, kind="ExternalInput")
    beta = nc.dram_tensor("beta", [D], F32, kind="ExternalInput")
    y_s = nc.dram_tensor("y_s", [B * TOK, D], F32, kind="ExternalOutput")

    with tile.TileContext(nc) as tc:
        with (
            tc.tile_pool(name="persist", bufs=1) as sbp,
            tc.tile_pool(name="small", bufs=2) as sbs,
            tc.tile_pool(name="dram", bufs=1, space="DRAM") as dram,
        ):
            # ---- persistent SBUF
            xT = sbp.tile([128, KC, NT], BF16)
            w1sb = sbp.tile([128, KC, 512], BF16)
            w2sb = sbp.tile([128, KC, D], BF16)
            kT = sbp.tile([128, NT], BF16)
            vT = sbp.tile([128, NT], BF16)
            qT = sbp.tile([128, NT], BF16)
            uT = sbp.tile([128, NT], BF16)
            vtok = sbp.tile([128, 2 * NKT, 128], BF16)
            gatedT = sbp.tile([128, NT], BF16)
            gfull = sbp.tile([128, KC, B * TOK], BF16)
            normed = sbp.tile([128, KC, B * TOK], BF16)

            # ---- input DMAs (order: first-needed first, fine-grained so the
            # first f1 matmuls start after ~1MB instead of ~12MB)
            b1sb = sbp.tile([128, 4], F32)
            nc.sync.dma_start(b1sb[:], b1c[:].rearrange("c p -> p c"))
            gam = sbp.tile([128, KC], F32)
            bet = sbp.tile([128, KC], F32)
            nc.sync.dma_start(gam[:], gamma[:].rearrange("(c p) -> p c", p=128))
            nc.sync.dma_start(bet[:], beta[:].rearrange("(c p) -> p c", p=128))
            b2_row = sbp.tile([1, D], F32R)
            nc.sync.dma_start(b2_row[:], b2[:][None, :])
            w1r = w1c[:].rearrange("(kc p) f -> p kc f", p=128)
            for kc in range(KC):
                nc.sync.dma_start(w1sb[:, kc, :], w1r[:, kc, :])
                nc.sync.dma_start(xT[:, kc, 0:2048], x_t[kc * 128:(kc + 1) * 128, 0:2048])
            for kc in range(KC):
                nc.sync.dma_start(xT[:, kc, 2048:NT],
                                  x_t[kc * 128:(kc + 1) * 128, 2048:NT])
            nc.sync.dma_start(w2sb[:], w2[:].rearrange("(kc p) f -> p kc f", p=128))

            # ---- constants
            ident_bf = sbp.tile([128, 128], BF16)
            make_identity(nc, ident_bf[:])
            ones_f = sbp.tile([128, 128], F32)
            nc.vector.memset(ones_f[:], 1.0)
            ones_col_bf = sbp.tile([128, 1], BF16)
            nc.vector.tensor_copy(ones_col_bf[:], ones_f[:, 0:1])
            ones_row_r = sbp.tile([1, 128], F32R)
            nc.vector.tensor_copy(ones_row_r[:], ones_f[0:1, :])

            # broadcast b2 across partitions via K=1 ones matmul
            b2sb = sbp.tile([128, D], F32)
            with tc.tile_pool(name="ps_bc", bufs=2, space="PSUM") as ps_bc:
                for nf in range(2):
                    pb = ps_bc.tile([128, 512], F32, tag="bc")
                    nc.tensor.matmul(pb[:], ones_row_r[:],
                                     b2_row[:, nf * 512:(nf + 1) * 512],
                                     start=True, stop=True)
                    nc.vector.tensor_copy(b2sb[:, nf * 512:(nf + 1) * 512], pb[:])

            # a2a DRAM bounce buffers, one per (batch, half). Core d owns
            # tokens [d*128, d*128+128) of each 1024-token half of each
            # batch, so the exchange for a half can fire as soon as its two
            # q-chunks are gated (overlapping the rest of attention).
            a2a_in = [dram.tile([D, 128], BF16, name=f"a2a_in{i}")
                      for i in range(2 * B)]
            a2a_out = [dram.tile([D, 128], BF16, name=f"a2a_out{i}")
                       for i in range(2 * B)]

            # ================= f1: k, v (feature-major, all 4096 tokens) ====
            PROJ_OUT = [kT, vT, qT, uT]

            def f1_grp(ps_pool, proj, grp):
                ps = ps_pool.tile([128, 2048], F32, tag="f1")
                for kc in range(KC):
                    lhsT = w1sb[:, kc, proj * 128:(proj + 1) * 128]
                    for t4 in range(4):
                        o = grp * 2048 + t4 * 512
                        nc.tensor.matmul(ps[:, t4 * 512:(t4 + 1) * 512],
                                         lhsT, xT[:, kc, o:o + 512],
                                         start=(kc == 0), stop=(kc == KC - 1))
                nc.scalar.activation(
                    PROJ_OUT[proj][:, grp * 2048:(grp + 1) * 2048], ps[:], SILU,
                    bias=b1sb[:, proj:proj + 1], scale=1.0)

            with tc.tile_pool(name="ps_f1a", bufs=2, space="PSUM") as ps_f1:
                for proj in (0, 1):           # k, v
                    for grp in (0, 1):
                        f1_grp(ps_f1, proj, grp)

            # ================= v transpose: [f, tok] -> [tok, f] =============
            with tc.tile_pool(name="ps_tr", bufs=4, space="PSUM") as ps_tr:
                for i in range(2 * NKT):
                    pt = ps_tr.tile([128, 128], BF16, tag="tr")
                    nc.tensor.transpose(pt[:], vT[:, i * 128:(i + 1) * 128],
                                        ident_bf[:])
                    nc.vector.tensor_copy(vtok[:, i, :], pt[:])

            # ================= f1: q, u =====================================
            with tc.tile_pool(name="ps_f1b", bufs=2, space="PSUM") as ps_f1b:
                for proj in (2, 3):           # q, u
                    for grp in (0, 1):
                        f1_grp(ps_f1b, proj, grp)

            # ================= attention per batch ==========================
            def ship_half(b, h):
                # dest d gets tokens [d*128, (d+1)*128) of half h of batch b
                idx = b * 2 + h
                for d in range(8):
                    o = b * S + h * 1024 + d * 128
                    nc.gpsimd.dma_start(a2a_in[idx][d * 128:(d + 1) * 128, :],
                                        gatedT[:, o:o + 128])
                nc.gpsimd.collective_compute(
                    "AllToAll", mybir.AluOpType.bypass,
                    replica_groups=[[0, 1, 2, 3, 4, 5, 6, 7]],
                    ins=[a2a_in[idx][:]], outs=[a2a_out[idx][:]])
                # readback: block s holds features [s*128,(s+1)*128) of my
                # 128 tokens (overlaps remaining attention work)
                to = b * TOK + h * 128
                for sblk in range(8):
                    nc.sync.dma_start(
                        gfull[:, sblk, to:to + 128],
                        a2a_out[idx][sblk * 128:(sblk + 1) * 128, :])

            with (
                tc.tile_pool(name="attn_a", bufs=3) as attn_a,
                tc.tile_pool(name="ps_s", bufs=2, space="PSUM") as ps_s,
                tc.tile_pool(name="ps_av", bufs=4, space="PSUM") as ps_av,
            ):
                for b in range(B):
                    boff = b * S
                    for qc in range(4):
                        qsl = boff + qc * 512
                        av = ps_av.tile([128, 512], F32, tag="av")
                        s_tiles = {}
                        a_tiles = {}

                        def emit_s(ktc):
                            s = ps_s.tile([128, 1024], F32, tag="s")
                            ksl = boff + ktc * 128
                            nc.tensor.matmul(s[:, 0:512],
                                             kT[0:64, ksl:ksl + 128],
                                             qT[0:64, qsl:qsl + 512],
                                             start=True, stop=True)
                            nc.tensor.matmul(s[:, 512:1024],
                                             kT[64:128, ksl:ksl + 128],
                                             qT[64:128, qsl:qsl + 512],
                                             start=True, stop=True,
                                             tile_position=(64, 0))
                            a = attn_a.tile([128, 1024], BF16, tag="a")
                            nc.scalar.activation(a[:], s[:], SILU)
                            a_tiles[ktc] = a

                        def emit_av(ktc):
                            a = a_tiles.pop(ktc)
                            vc = b * NKT + ktc
                            nc.tensor.matmul(av[0:64, :], vtok[:, vc, 0:64],
                                             a[:, 0:512],
                                             start=(ktc == 0), stop=(ktc == NKT - 1))
                            nc.tensor.matmul(av[64:128, :], vtok[:, vc, 64:128],
                                             a[:, 512:1024],
                                             start=(ktc == 0), stop=(ktc == NKT - 1),
                                             tile_position=(0, 64))

                        for ktc in range(NKT):
                            emit_s(ktc)
                            if ktc >= 1:
                                emit_av(ktc - 1)
                        emit_av(NKT - 1)

                        nc.vector.tensor_tensor(gatedT[:, qsl:qsl + 512], av[:],
                                                uT[:, qsl:qsl + 512], MULT)

                        if qc == 1:
                            ship_half(b, 0)
                        elif qc == 3:
                            ship_half(b, 1)

            # ================= LayerNorm + f2 per batch =====================
            with (
                tc.tile_pool(name="ln", bufs=2) as ln,
                tc.tile_pool(name="sqp", bufs=1) as sqp,
                tc.tile_pool(name="yout", bufs=2) as yout,
                tc.tile_pool(name="ps_ln", bufs=1, space="PSUM") as ps_ln,
                tc.tile_pool(name="ps_y", bufs=2, space="PSUM") as ps_y,
            ):
                for b in range(B):
                    bsl = slice(b * TOK, (b + 1) * TOK)
                    sq = sqp.tile([128, KC, TOK], BF16, tag="sq")
                    for kc in range(KC):
                        # gpsimd (idle) keeps this off the DVE queue, whose
                        # in-order stream still has attention gating work
                        nc.gpsimd.tensor_tensor(sq[:, kc, :], gfull[:, kc, bsl],
                                                gfull[:, kc, bsl], MULT)
                    st_sum = ps_ln.tile([1, TOK], F32, tag="st_sum")
                    st_sq = ps_ln.tile([1, TOK], F32, tag="st_sq")
                    for kc in range(KC):
                        nc.tensor.matmul(st_sum[:], ones_col_bf[:],
                                         gfull[:, kc, bsl],
                                         start=(kc == 0), stop=(kc == KC - 1))
                    for kc in range(KC):
                        nc.tensor.matmul(st_sq[:], ones_col_bf[:], sq[:, kc, :],
                                         start=(kc == 0), stop=(kc == KC - 1))

                    mu = ln.tile([1, TOK], F32, tag="mu")
                    nc.vector.tensor_scalar_mul(mu[:], st_sum[:], 1.0 / D)
                    m2 = ln.tile([1, TOK], F32, tag="m2")
                    nc.vector.tensor_scalar_mul(m2[:], st_sq[:], 1.0 / D)
                    mu2 = ln.tile([1, TOK], F32, tag="mu2")
                    nc.vector.tensor_tensor(mu2[:], mu[:], mu[:], MULT)
                    varE = ln.tile([1, TOK], F32, tag="varE")
                    nc.vector.tensor_tensor(varE[:], m2[:], mu2[:], SUB)
                    nc.vector.tensor_scalar_add(varE[:], varE[:], EPS_EFF)
                    std = ln.tile([1, TOK], F32, tag="std")
                    nc.scalar.activation(std[:], varE[:], SQRT)
                    r0 = ln.tile([1, TOK], F32, tag="r0")
                    nc.vector.reciprocal(r0[:], std[:])
                    # one Newton step: r1 = r0 * (1.5 - 0.5 * varE * r0^2)
                    nt1 = ln.tile([1, TOK], F32, tag="nt1")
                    nc.vector.tensor_tensor(nt1[:], r0[:], r0[:], MULT)
                    nc.vector.tensor_tensor(nt1[:], nt1[:], varE[:], MULT)
                    nc.vector.tensor_scalar(nt1[:], nt1[:], -0.5, 1.5, MULT, ADD)
                    rstd = ln.tile([1, TOK], F32R, tag="rstd")
                    nc.vector.tensor_tensor(rstd[:], r0[:], nt1[:], MULT)
                    mu_r = ln.tile([1, TOK], F32R, tag="mu_r")
                    nc.vector.tensor_copy(mu_r[:], mu[:])

                    ps_mu = ps_ln.tile([128, TOK], F32, tag="ps_mu")
                    ps_r = ps_ln.tile([128, TOK], F32, tag="ps_r")
                    nc.tensor.matmul(ps_mu[:], ones_row_r[:], mu_r[:],
                                     start=True, stop=True)
                    nc.tensor.matmul(ps_r[:], ones_row_r[:], rstd[:],
                                     start=True, stop=True)

                    for kc in range(KC):
                        t1 = ln.tile([128, TOK], F32, tag="t1")
                        nc.vector.tensor_tensor(t1[:], gfull[:, kc, bsl],
                                                ps_mu[:], SUB)
                        nc.vector.tensor_tensor(t1[:], t1[:], ps_r[:], MULT)
                        nc.vector.tensor_scalar(normed[:, kc, bsl], t1[:],
                                                gam[:, kc:kc + 1],
                                                bet[:, kc:kc + 1], MULT, ADD)

                    # f2: y[tok, :] = normed^T @ W2 + b2
                    for tt in range(TOK // 128):
                        for nf in range(2):
                            psy = ps_y.tile([128, 512], F32, tag="y")
                            for kc in range(KC):
                                nc.tensor.matmul(
                                    psy[:],
                                    normed[:, kc, b * TOK + tt * 128:
                                           b * TOK + (tt + 1) * 128],
                                    w2sb[:, kc, nf * 512:(nf + 1) * 512],
                                    start=(kc == 0), stop=(kc == KC - 1))
                            yo = yout.tile([128, 512], F32, tag="yo")
                            nc.vector.tensor_tensor(
                                yo[:], psy[:], b2sb[:, nf * 512:(nf + 1) * 512],
                                ADD)
                            nc.sync.dma_start(
                                y_s[b * TOK + tt * 128: b * TOK + (tt + 1) * 128,
                                    nf * 512:(nf + 1) * 512], yo[:])

    nc.compile()
    return nc


def _get_nc():
    if "nc" not in _CACHE:
        _CACHE["nc"] = _build()
    return _CACHE["nc"]


def kernel(x, W1, b1, W2, b2, gamma, beta, **kw):
    nc = _get_nc()
    x = np.ascontiguousarray(x, dtype=np.float32)
    W1 = np.ascontiguousarray(W1, dtype=np.float32)
    U0, V0, Q0, K0 = 0, D, 2 * D, 3 * D

    x2 = np.concatenate([x[0], x[1]], axis=0)            # [4096, 1024]
    xt = np.ascontiguousarray(x2.T.astype(BF))           # [1024, 4096]
    w2_bf = np.ascontiguousarray(W2.astype(BF))

    in_maps = []
    for c in range(8):
        sl = slice(c * 128, (c + 1) * 128)
        w1c = np.ascontiguousarray(np.concatenate(
            [W1[:, K0 + c * 128:K0 + (c + 1) * 128],
             W1[:, V0 + c * 128:V0 + (c + 1) * 128],
             W1[:, Q0 + c * 128:Q0 + (c + 1) * 128],
             W1[:, U0 + c * 128:U0 + (c + 1) * 128]], axis=1).astype(BF))
        b1cc = np.ascontiguousarray(np.stack(
            [b1[K0 + c * 128:K0 + (c + 1) * 128],
             b1[V0 + c * 128:V0 + (c + 1) * 128],
             b1[Q0 + c * 128:Q0 + (c + 1) * 128],
             b1[U0 + c * 128:U0 + (c + 1) * 128]], axis=0).astype(np.float32))
        in_maps.append({
            "x_t": xt,
            "w1c": w1c,
            "b1c": b1cc,
            "w2": w2_bf,
            "b2": np.ascontiguousarray(b2, dtype=np.float32),
            "gamma": np.ascontiguousarray(gamma, dtype=np.float32),
            "beta": np.ascontiguousarray(beta, dtype=np.float32),
        })
    res = run_bass_kernel_spmd(nc, in_maps, core_ids=list(range(8)), **kw)
    y = np.empty((B, S, D), dtype=np.float32)
    for c in range(8):
        for b in range(B):
            for h in range(2):
                y[b, h * 1024 + c * 128: h * 1024 + (c + 1) * 128, :] = \
                    res.results[c]["y_s"][b * TOK + h * 128: b * TOK + (h + 1) * 128]
    if kw:
        _CACHE["last_res"] = res
    return y


# revision 13
# speedup vs baseline: 2.0520x; 1.1130x over previous
"""HSTU block kernel for 8 Trainium2 NeuronCores — head-sharded (tensor parallel).

Sharding: core c owns heads {2c, 2c+1} (feature slice c*128:(c+1)*128 of each
of u/v/q/k) for BOTH batches, computing f1 + attention + gating for all 4096
tokens on its slice. A per-batch AllToAll (512KB) then redistributes the gated
activations token-wise (core c owns tokens [256c, 256c+256) of each batch) for
LayerNorm + f2. This replaces the old token-sharded design's 8MB k/v
AllGather (measured 128us on the ring) with 2x ~15us exchanges.

Everything runs in bf16 (PE rate on TRN2 is 1 row/cycle for bf16 and fp32r
alike, but bf16 halves DMA + LDWEIGHTS). x is pre-transposed and pre-cast on
the host, so the device does zero input transposes; v is transposed
feature->token major on the PE (32 small transposes) for the AV matmul.

silu(scores)/S scaling is folded into LayerNorm via eps' = S^2 * eps
(LN is scale-invariant except for eps). All activations (silu) run on the
ACT engine; LN's rsqrt uses ACT sqrt + DVE reciprocal + 1 Newton step,
placed after all silu work so the activation table loads only once.
"""

import sys

sys.path.insert(0, "/opt/trn_rl_repo")

import ml_dtypes
import numpy as np

import concourse.bass as bass
import concourse.mybir as mybir
import concourse.tile as tile
from concourse import bacc
from concourse.bass_utils import run_bass_kernel_spmd
from concourse.masks import make_identity

F32 = mybir.dt.float32
F32R = mybir.dt.float32r
BF16 = mybir.dt.bfloat16
SILU = mybir.ActivationFunctionType.Silu
SQRT = mybir.ActivationFunctionType.Sqrt
MULT = mybir.AluOpType.mult
ADD = mybir.AluOpType.add
SUB = mybir.AluOpType.subtract

B, S, D = 2, 2048, 1024
NT = B * S          # 4096 tokens total (both batches)
TOK = 256           # owned tokens per batch per core
KC = D // 128       # 8 contraction chunks
NKT = S // 128      # 16 k-token chunks per batch
EPS_EFF = float(S) * float(S) * 1e-5
BF = ml_dtypes.bfloat16

_CACHE = {}


def _build():
    nc = bacc.Bacc(None, target_bir_lowering=False, num_devices=8)

    x_t = nc.dram_tensor("x_t", [D, NT], BF16, kind="ExternalInput")
    w1c = nc.dram_tensor("w1c", [D, 512], BF16, kind="ExternalInput")
    b1c = nc.dram_tensor("b1c", [4, 128], F32, kind="ExternalInput")
    w2 = nc.dram_tensor("w2", [D, D], BF16, kind="ExternalInput")
    b2 = nc.dram_tensor("b2", [D], F32R, kind="ExternalInput")
    gamma = nc.dram_tensor("gamma", [D], F32, kind="ExternalInput")
    beta = nc.dram_tensor("beta", [D], F32, kind="ExternalInput")
    y_s = nc.dram_tensor("y_s", [B * TOK, D], F32, kind="ExternalOutput")

    with tile.TileContext(nc) as tc:
        with (
            tc.tile_pool(name="persist", bufs=1) as sbp,
            tc.tile_pool(name="small", bufs=2) as sbs,
            tc.tile_pool(name="dram", bufs=1, space="DRAM") as dram,
        ):
            # ---- persistent SBUF
            xT = sbp.tile([128, KC, NT], BF16)
            w1sb = sbp.tile([128, KC, 512], BF16)
            w2sb = sbp.tile([128, KC, D], BF16)
            kT = sbp.tile([128, NT], BF16)
            vT = sbp.tile([128, NT], BF16)
            qT = sbp.tile([128, NT], BF16)
            uT = sbp.tile([128, NT], BF16)
            vtok = sbp.tile([128, 2 * NKT, 128], BF16)
            gatedT = sbp.tile([128, NT], BF16)
            gfull = sbp.tile([128, KC, B * TOK], BF16)
            normed = sbp.tile([128, KC, B * TOK], BF16)

            # ---- input DMAs (order: first-needed first, fine-grained so the
            # first f1 matmuls start after ~1MB instead of ~12MB)
            b1sb = sbp.tile([128, 4], F32)
            nc.sync.dma_start(b1sb[:], b1c[:].rearrange("c p -> p c"))
            gam = sbp.tile([128, KC], F32)
            bet = sbp.tile([128, KC], F32)
            nc.sync.dma_start(gam[:], gamma[:].rearrange("(c p) -> p c", p=128))
            nc.sync.dma_start(bet[:], beta[:].rearrange("(c p) -> p c", p=128))
            b2_row = sbp.tile([1, D], F32R)
            nc.sync.dma_start(b2_row[:], b2[:][None, :])
            w1r = w1c[:].rearrange("(kc p) f -> p kc f", p=128)
            for kc in range(KC):
                nc.sync.dma_start(w1sb[:, kc, :], w1r[:, kc, :])
                nc.sync.dma_start(xT[:, kc, 0:2048], x_t[kc * 128:(kc + 1) * 128, 0:2048])
            for kc in range(KC):
                nc.sync.dma_start(xT[:, kc, 2048:NT],
                                  x_t[kc * 128:(kc + 1) * 128, 2048:NT])
            nc.sync.dma_start(w2sb[:], w2[:].rearrange("(kc p) f -> p kc f", p=128))

            # ---- constants
            ident_bf = sbp.tile([128, 128], BF16)
            make_identity(nc, ident_bf[:])
            ones_f = sbp.tile([128, 128], F32)
            nc.vector.memset(ones_f[:], 1.0)
            ones_col_bf = sbp.tile([128, 1], BF16)
            nc.vector.tensor_copy(ones_col_bf[:], ones_f[:, 0:1])
            ones_row_r = sbp.tile([1, 128], F32R)
            nc.vector.tensor_copy(ones_row_r[:], ones_f[0:1, :])

            # broadcast b2 across partitions via K=1 ones matmul
            b2sb = sbp.tile([128, D], F32)
            with tc.tile_pool(name="ps_bc", bufs=2, space="PSUM") as ps_bc:
                for nf in range(2):
                    pb = ps_bc.tile([128, 512], F32, tag="bc")
                    nc.tensor.matmul(pb[:], ones_row_r[:],
                                     b2_row[:, nf * 512:(nf + 1) * 512],
                                     start=True, stop=True)
                    nc.vector.tensor_copy(b2sb[:, nf * 512:(nf + 1) * 512], pb[:])

            # a2a DRAM bounce buffers, one per (batch, half). Core d owns
            # tokens [d*128, d*128+128) of each 1024-token half of each
            # batch, so the exchange for a half can fire as soon as its two
            # q-chunks are gated (overlapping the rest of attention).
            a2a_in = [dram.tile([D, 128], BF16, name=f"a2a_in{i}")
                      for i in range(2 * B)]
            a2a_out = [dram.tile([D, 128], BF16, name=f"a2a_out{i}")
                       for i in range(2 * B)]

            # ================= f1: k, v (feature-major, all 4096 tokens) ====
            PROJ_OUT = [kT, vT, qT, uT]

            def f1_grp(ps_pool, proj, grp):
                ps = ps_pool.tile([128, 2048], F32, tag="f1")
                for kc in range(KC):
                    lhsT = w1sb[:, kc, proj * 128:(proj + 1) * 128]
                    for t4 in range(4):
                        o = grp * 2048 + t4 * 512
                        nc.tensor.matmul(ps[:, t4 * 512:(t4 + 1) * 512],
                                         lhsT, xT[:, kc, o:o + 512],
                                         start=(kc == 0), stop=(kc == KC - 1))
                nc.scalar.activation(
                    PROJ_OUT[proj][:, grp * 2048:(grp + 1) * 2048], ps[:], SILU,
                    bias=b1sb[:, proj:proj + 1], scale=1.0)

            with tc.tile_pool(name="ps_f1a", bufs=2, space="PSUM") as ps_f1:
                for proj in (0, 1):           # k, v
                    for grp in (0, 1):
                        f1_grp(ps_f1, proj, grp)

            # ================= v transpose: [f, tok] -> [tok, f] =============
            with tc.tile_pool(name="ps_tr", bufs=8, space="PSUM") as ps_tr:
                for i in range(2 * NKT):
                    pt = ps_tr.tile([128, 128], BF16, tag="tr")
                    nc.tensor.transpose(pt[:], vT[:, i * 128:(i + 1) * 128],
                                        ident_bf[:])
                    # alternate drain engines so the PE isn't paced by one
                    if i % 2 == 0:
                        nc.vector.tensor_copy(vtok[:, i, :], pt[:])
                    else:
                        nc.scalar.copy(vtok[:, i, :], pt[:])

            # ================= f1: q, u =====================================
            with tc.tile_pool(name="ps_f1b", bufs=2, space="PSUM") as ps_f1b:
                for proj in (2, 3):           # q, u
                    for grp in (0, 1):
                        f1_grp(ps_f1b, proj, grp)

            # ================= attention per batch ==========================
            def ship_half(b, h):
                # dest d gets tokens [d*128, (d+1)*128) of half h of batch b
                idx = b * 2 + h
                for d in range(8):
                    o = b * S + h * 1024 + d * 128
                    nc.gpsimd.dma_start(a2a_in[idx][d * 128:(d + 1) * 128, :],
                                        gatedT[:, o:o + 128])
                nc.gpsimd.collective_compute(
                    "AllToAll", mybir.AluOpType.bypass,
                    replica_groups=[[0, 1, 2, 3, 4, 5, 6, 7]],
                    ins=[a2a_in[idx][:]], outs=[a2a_out[idx][:]])
                # readback: block s holds features [s*128,(s+1)*128) of my
                # 128 tokens (overlaps remaining attention work)
                to = b * TOK + h * 128
                for sblk in range(8):
                    nc.sync.dma_start(
                        gfull[:, sblk, to:to + 128],
                        a2a_out[idx][sblk * 128:(sblk + 1) * 128, :])

            with (
                tc.tile_pool(name="attn_a", bufs=3) as attn_a,
                tc.tile_pool(name="ps_s", bufs=3, space="PSUM") as ps_s,
                tc.tile_pool(name="ps_av", bufs=2, space="PSUM") as ps_av,
            ):
                for b in range(B):
                    boff = b * S
                    for qc in range(4):
                        qsl = boff + qc * 512
                        av = ps_av.tile([128, 512], F32, tag="av")
                        s_tiles = {}
                        a_tiles = {}

                        def emit_s(ktc):
                            s = ps_s.tile([128, 1024], F32, tag="s")
                            ksl = boff + ktc * 128
                            nc.tensor.matmul(s[:, 0:512],
                                             kT[0:64, ksl:ksl + 128],
                                             qT[0:64, qsl:qsl + 512],
                                             start=True, stop=True)
                            nc.tensor.matmul(s[:, 512:1024],
                                             kT[64:128, ksl:ksl + 128],
                                             qT[64:128, qsl:qsl + 512],
                                             start=True, stop=True,
                                             tile_position=(64, 0))
                            a = attn_a.tile([128, 1024], BF16, tag="a")
                            nc.scalar.activation(a[:], s[:], SILU)
                            a_tiles[ktc] = a

                        def emit_av(ktc):
                            a = a_tiles.pop(ktc)
                            vc = b * NKT + ktc
                            nc.tensor.matmul(av[0:64, :], vtok[:, vc, 0:64],
                                             a[:, 0:512],
                                             start=(ktc == 0), stop=(ktc == NKT - 1))
                            nc.tensor.matmul(av[64:128, :], vtok[:, vc, 64:128],
                                             a[:, 512:1024],
                                             start=(ktc == 0), stop=(ktc == NKT - 1),
                                             tile_position=(0, 64))

                        for ktc in range(NKT):
                            emit_s(ktc)
                            if ktc >= 1:
                                emit_av(ktc - 1)
                        emit_av(NKT - 1)

                        nc.vector.tensor_tensor(gatedT[:, qsl:qsl + 512], av[:],
                                                uT[:, qsl:qsl + 512], MULT)

                        if qc == 1:
                            ship_half(b, 0)
                        elif qc == 3:
                            ship_half(b, 1)

            # ================= LayerNorm + f2 per batch =====================
            with (
                tc.tile_pool(name="ln", bufs=2) as ln,
                tc.tile_pool(name="sqp", bufs=1) as sqp,
                tc.tile_pool(name="yout", bufs=2) as yout,
                tc.tile_pool(name="ps_ln", bufs=1, space="PSUM") as ps_ln,
                tc.tile_pool(name="ps_y", bufs=2, space="PSUM") as ps_y,
            ):
                # one LN+f2 block per 128-token half: the last a2a's wait is
                # covered by the three halves whose data already arrived
                for b in range(B):
                    for h in range(2):
                        to = b * TOK + h * 128
                        bsl = slice(to, to + 128)
                        sq = sqp.tile([128, KC, 128], BF16, tag="sq")
                        for kc in range(KC):
                            # gpsimd (idle) keeps this off the DVE queue,
                            # whose in-order stream still has gating work
                            nc.gpsimd.tensor_tensor(sq[:, kc, :],
                                                    gfull[:, kc, bsl],
                                                    gfull[:, kc, bsl], MULT)
                        st_sum = ps_ln.tile([1, 128], F32, tag="st_sum")
                        st_sq = ps_ln.tile([1, 128], F32, tag="st_sq")
                        for kc in range(KC):
                            nc.tensor.matmul(st_sum[:], ones_col_bf[:],
                                             gfull[:, kc, bsl],
                                             start=(kc == 0), stop=(kc == KC - 1))
                        for kc in range(KC):
                            nc.tensor.matmul(st_sq[:], ones_col_bf[:],
                                             sq[:, kc, :],
                                             start=(kc == 0), stop=(kc == KC - 1))

                        mu = ln.tile([1, 128], F32, tag="mu")
                        nc.vector.tensor_scalar_mul(mu[:], st_sum[:], 1.0 / D)
                        m2 = ln.tile([1, 128], F32, tag="m2")
                        nc.vector.tensor_scalar_mul(m2[:], st_sq[:], 1.0 / D)
                        mu2 = ln.tile([1, 128], F32, tag="mu2")
                        nc.vector.tensor_tensor(mu2[:], mu[:], mu[:], MULT)
                        varE = ln.tile([1, 128], F32, tag="varE")
                        nc.vector.tensor_tensor(varE[:], m2[:], mu2[:], SUB)
                        nc.vector.tensor_scalar_add(varE[:], varE[:], EPS_EFF)
                        std = ln.tile([1, 128], F32, tag="std")
                        nc.scalar.activation(std[:], varE[:], SQRT)
                        r0 = ln.tile([1, 128], F32, tag="r0")
                        nc.vector.reciprocal(r0[:], std[:])
                        # one Newton step: r1 = r0 * (1.5 - 0.5*varE*r0^2)
                        nt1 = ln.tile([1, 128], F32, tag="nt1")
                        nc.vector.tensor_tensor(nt1[:], r0[:], r0[:], MULT)
                        nc.vector.tensor_tensor(nt1[:], nt1[:], varE[:], MULT)
                        nc.vector.tensor_scalar(nt1[:], nt1[:], -0.5, 1.5,
                                                MULT, ADD)
                        rstd = ln.tile([1, 128], F32R, tag="rstd")
                        nc.vector.tensor_tensor(rstd[:], r0[:], nt1[:], MULT)
                        mu_r = ln.tile([1, 128], F32R, tag="mu_r")
                        nc.vector.tensor_copy(mu_r[:], mu[:])

                        ps_mu = ps_ln.tile([128, 128], F32, tag="ps_mu")
                        ps_r = ps_ln.tile([128, 128], F32, tag="ps_r")
                        nc.tensor.matmul(ps_mu[:], ones_row_r[:], mu_r[:],
                                         start=True, stop=True)
                        nc.tensor.matmul(ps_r[:], ones_row_r[:], rstd[:],
                                         start=True, stop=True)

                        for kc in range(KC):
                            t1 = ln.tile([128, 128], F32, tag="t1")
                            nc.vector.tensor_tensor(t1[:], gfull[:, kc, bsl],
                                                    ps_mu[:], SUB)
                            nc.vector.tensor_tensor(t1[:], t1[:], ps_r[:], MULT)
                            nc.vector.tensor_scalar(normed[:, kc, bsl], t1[:],
                                                    gam[:, kc:kc + 1],
                                                    bet[:, kc:kc + 1], MULT, ADD)

                        # f2: y[tok, :] = normed^T @ W2 + b2
                        for nf in range(2):
                            psy = ps_y.tile([128, 512], F32, tag="y")
                            for kc in range(KC):
                                nc.tensor.matmul(
                                    psy[:], normed[:, kc, bsl],
                                    w2sb[:, kc, nf * 512:(nf + 1) * 512],
                                    start=(kc == 0), stop=(kc == KC - 1))
                            yo = yout.tile([128, 512], F32, tag="yo")
                            nc.vector.tensor_tensor(
                                yo[:], psy[:], b2sb[:, nf * 512:(nf + 1) * 512],
                                ADD)
                            nc.sync.dma_start(
                                y_s[to:to + 128, nf * 512:(nf + 1) * 512],
                                yo[:])

    nc.compile()
    return nc


def _get_nc():
    if "nc" not in _CACHE:
        _CACHE["nc"] = _build()
    return _CACHE["nc"]


def kernel(x, W1, b1, W2, b2, gamma, beta, **kw):
    nc = _get_nc()
    x = np.ascontiguousarray(x, dtype=np.float32)
    W1 = np.ascontiguousarray(W1, dtype=np.float32)
    U0, V0, Q0, K0 = 0, D, 2 * D, 3 * D

    x2 = np.concatenate([x[0], x[1]], axis=0)            # [4096, 1024]
    xt = np.ascontiguousarray(x2.T.astype(BF))           # [1024, 4096]
    w2_bf = np.ascontiguousarray(W2.astype(BF))

    in_maps = []
    for c in range(8):
        sl = slice(c * 128, (c + 1) * 128)
        w1c = np.ascontiguousarray(np.concatenate(
            [W1[:, K0 + c * 128:K0 + (c + 1) * 128],
             W1[:, V0 + c * 128:V0 + (c + 1) * 128],
             W1[:, Q0 + c * 128:Q0 + (c + 1) * 128],
             W1[:, U0 + c * 128:U0 + (c + 1) * 128]], axis=1).astype(BF))
        b1cc = np.ascontiguousarray(np.stack(
            [b1[K0 + c * 128:K0 + (c + 1) * 128],
             b1[V0 + c * 128:V0 + (c + 1) * 128],
             b1[Q0 + c * 128:Q0 + (c + 1) * 128],
             b1[U0 + c * 128:U0 + (c + 1) * 128]], axis=0).astype(np.float32))
        in_maps.append({
            "x_t": xt,
            "w1c": w1c,
            "b1c": b1cc,
            "w2": w2_bf,
            "b2": np.ascontiguousarray(b2, dtype=np.float32),
            "gamma": np.ascontiguousarray(gamma, dtype=np.float32),
            "beta": np.ascontiguousarray(beta, dtype=np.float32),
        })
    res = run_bass_kernel_spmd(nc, in_maps, core_ids=list(range(8)), **kw)
    y = np.empty((B, S, D), dtype=np.float32)
    for c in range(8):
        for b in range(B):
            for h in range(2):
                y[b, h * 1024 + c * 128: h * 1024 + (c + 1) * 128, :] = \
                    res.results[c]["y_s"][b * TOK + h * 128: b * TOK + (h + 1) * 128]
    if kw:
        _CACHE["last_res"] = res
    return y
